# revision 33
# baseline (speedup 1.0000x reference)
"""Trainium2 Bass kernel for HalfHadamardTrustQuantizer.

Computation (forward value of the reference, which collapses to xq):
  x_had = blockwise-64 Hadamard rotation of channels:  (B,C,H,W), C=512 = 8 groups of 64
  std   = sqrt(mean(x_had^2)) per sample  (== RMS of x by orthogonality)
  scale = OPT*std + 1e-8 ; step = 2*scale/15
  xq    = round(clip(x_had,-scale,scale)/step + 0.5)*step - step/2

Sharding: data-parallel over batch; 2 samples per core on 8 cores.

Pipeline (per core, 2 samples of (512, 3136) fp32; ~78us vs 113us
baseline; wire-limited: 25.7MB on one DMA queue at ~420-430 GB/s):
  ALL x/y DMA on the single Pool SWDGE queue, strict FIFO: 16 casting
  half-row loads (f32 -> f32r, sample 0 first, split at col 2048 to
  match PSUM drain tiles) followed by 8 full-row casting stores
  (bf16 -> f32).  One stream sustains ~420-430 GB/s; stores begin
  right as loads end; wt loads on the idle Sync HWDGE queue + a DVE
  convert.  A dummy [128,1] Sqrt is the first ACT op so the act-table
  pass pins the sqrt_and_others table (square+identity+sqrt) - no
  1.3us table reloads mid-kernel.
  Input side: ACT squares the 2048-halves, DVE squares the 1088-halves
  (accum_out partial columns).  Scalars: DVE reduce + PE ones-matmul
  broadcast + ACT sqrt + DVE ops; sample 1's tot PSUM slot is reserved
  mid-rotation so its scalar chain is not gated by the full s0 drain
  pipeline.
  Phase B per row: PE f32r matmuls into 2 PSUM tiles (2048/1088); ACT
  drains the 2048 tile (Identity, scale=1/step, bias=0.5 -> i16 RNE),
  the 1088 tile drains on DVE (s0 rows) / ACT (s1 rows); DVE affine
  i16 -> bf16 (2x rate), then ONE full-row bf16 clip on DVE against
  +-scale AP bounds (clip(i,-7,8)*step-step/2 == clip(i*step-step/2,
  -scale,scale), so the int clip becomes a value clip after the
  affine); full-row casting store bf16 -> f32 on Q0.  Pool runs no
  elementwise work - only DMA issues - keeping the drain->store chain
  on ACT+DVE.  Sample 1's squares are emitted after sample 0's phase B
  so the scheduler fills engine gaps with them instead of blocking the
  chain.  bf16 output costs ~0.1% extra rounding (budget 2e-2).
"""

import numpy as np
from contextlib import ExitStack

B, C, HH, WW = 16, 512, 56, 56
R = HH * WW            # 3136 spatial
NCORES = 8
S = B // NCORES        # samples per core
NB = C // 128          # block-rows per sample
N_ELEM = C * R         # per-sample reduction size
OPT = 2.513930578568423
INV_N = np.float64(1.0) / np.float64(N_ELEM)
TWO_15 = np.float32(2.0) / np.float32(15.0)

WA = 2048              # A-half width (cols 0:2048)
WB = R - WA            # B-half width (cols 2048:3136) = 1088

_CACHE = {}


def _build_program():
    import concourse.bacc as bacc
    import concourse.tile as tile
    import concourse.mybir as mybir

    AF = mybir.ActivationFunctionType
    OP = mybir.AluOpType
    f32 = mybir.dt.float32
    f32r = mybir.dt.float32r
    bf16 = mybir.dt.bfloat16
    i16 = mybir.dt.int16

    nc = bacc.Bacc("TRN2", target_bir_lowering=False, debug=False,
                   num_devices=NCORES)
    x = nc.dram_tensor("x", [S * C, R], f32, kind="ExternalInput").ap()
    w = nc.dram_tensor("w", [128, 128], f32, kind="ExternalInput").ap()
    y = nc.dram_tensor("y", [S * C, R], f32, kind="ExternalOutput").ap()

    with tile.TileContext(nc) as tc, ExitStack() as ctx:
        xap = ctx.enter_context(tc.tile_pool(name="xap", bufs=2 * NB))
        xbp = ctx.enter_context(tc.tile_pool(name="xbp", bufs=2 * NB))
        cn = ctx.enter_context(tc.tile_pool(name="cn", bufs=1))
        sq = ctx.enter_context(tc.tile_pool(name="sq", bufs=1))
        ac = ctx.enter_context(tc.tile_pool(name="ac", bufs=2))
        sc = ctx.enter_context(tc.tile_pool(name="sc", bufs=2))
        iap = ctx.enter_context(tc.tile_pool(name="iap", bufs=3))
        ibp = ctx.enter_context(tc.tile_pool(name="ibp", bufs=3))
        op_ = ctx.enter_context(tc.tile_pool(name="op", bufs=3))
        pp = ctx.enter_context(tc.tile_pool(name="pp", bufs=2, space="PSUM"))

        # constants first (instant readiness for the dummy sqrt)
        ones = cn.tile([128, 128], f32, tag="ones")
        nc.gpsimd.memset(ones[:], 1.0)
        half = cn.tile([128, 1], f32, tag="half")
        nc.gpsimd.memset(half[:], 0.5)
        # dummy sqrt pins the sqrt_and_others act table (square+identity+sqrt)
        dum = cn.tile([128, 1], f32, tag="dum")
        nc.scalar.activation(dum[:], half[:], AF.Sqrt)

        # ---- wt on the idle Sync HWDGE queue; DVE converts to f32r ----
        wt_f = cn.tile([128, 128], f32, tag="wf")
        nc.sync.dma_start(wt_f[:], w[:])
        wt = cn.tile([128, 128], f32r, tag="w")
        nc.vector.tensor_scalar_mul(wt[:], wt_f[:], 1.0)
        # ---- Q0 SWDGE FIFO: 16 casting half-loads first, then stores ----

        xas = {}
        xbs = {}
        for s in range(S):
            for b in range(NB):
                r0 = s * C + b * 128
                xa = xap.tile([128, WA], f32r, tag="xa")
                nc.gpsimd.dma_start(xa[:], x[r0:r0 + 128, 0:WA])
                xas[(s, b)] = xa
                xb = xbp.tile([128, WB], f32r, tag="xb")
                nc.gpsimd.dma_start(xb[:], x[r0:r0 + 128, WA:R])
                xbs[(s, b)] = xb

        sqa = sq.tile([128, WA], f32, tag="sqa")   # ACT square scratch
        sqb = sq.tile([128, WB], f32, tag="sqb")   # DVE square scratch

        parts = {}
        for s in range(S):
            parts[s] = ac.tile([128, 2 * NB], f32, tag=f"part{s}",
                               name=f"part{s}")

        scal = {}

        def sq_a(s, b):
            nc.scalar.activation(sqa[:], xas[(s, b)][:].bitcast(f32),
                                 AF.Square,
                                 accum_out=parts[s][:, 2 * b:2 * b + 1])

        def sq_b(s, b):
            src = xbs[(s, b)][:].bitcast(f32)
            nc.vector.scalar_tensor_tensor(sqb[:], src, 1.0, src,
                                           OP.mult, OP.mult,
                                           accum_out=parts[s][:, 2 * b + 1:
                                                              2 * b + 2])

        def sample_scalars(s, tot_tile=None):
            red = sc.tile([128, 1], f32, tag="red")
            nc.vector.reduce_sum(red[:], parts[s][:], axis=mybir.AxisListType.X)
            tot = tot_tile if tot_tile is not None else pp.tile(
                [128, WA], f32, tag="pchunk")
            tot = tot[:, 0:1]
            nc.tensor.matmul(tot[:], ones[:], red[:], start=True, stop=True)
            std = sc.tile([128, 1], f32, tag="std")
            nc.scalar.activation(std[:], tot[:], AF.Sqrt, scale=float(INV_N))
            scale_t = sc.tile([128, 1], f32, tag="scale")
            nc.vector.tensor_scalar(scale_t[:], std[:], float(OPT), 1e-8,
                                    OP.mult, OP.add)
            step = sc.tile([128, 1], f32, tag="step")
            nc.vector.tensor_scalar_mul(step[:], scale_t[:], float(TWO_15))
            inv = sc.tile([128, 1], f32, tag="inv")
            nc.vector.reciprocal(inv[:], step[:])
            hstep = sc.tile([128, 1], f32, tag="hstep")
            nc.vector.tensor_scalar_mul(hstep[:], step[:], 0.5)
            nscale = sc.tile([128, 1], f32, tag="nscale")
            nc.vector.tensor_scalar_mul(nscale[:], scale_t[:], -1.0)
            scal[s] = (inv, step, hstep, scale_t, nscale)

        def phase_b_row(s, b):
            inv, step, hstep, scale_t, nscale = scal[s]
            xa = xas.pop((s, b))
            xb = xbs.pop((s, b))
            pma = pp.tile([128, WA], f32, tag="pchunk")
            for co in range(0, WA, 512):
                nc.tensor.matmul(pma[:, co:co + 512], wt[:],
                                 xa[:, co:co + 512], start=True, stop=True)
            pmb = pp.tile([128, WA], f32, tag="pchunk")
            for co in range(0, WB, 512):
                ch = min(512, WB - co)
                nc.tensor.matmul(pmb[:, co:co + ch], wt[:],
                                 xb[:, co:co + ch], start=True, stop=True)
            ia = iap.tile([128, WA], i16, tag="ia")
            nc.scalar.activation(ia[:], pma[:, :WA], AF.Identity,
                                 bias=half[:], scale=inv[:])
            ib = ibp.tile([128, WB], i16, tag="ib")
            if s == 0:
                nc.vector.tensor_scalar(ib[:], pmb[:, :WB], inv[:], half[:],
                                        OP.mult, OP.add)
            else:
                nc.scalar.activation(ib[:], pmb[:, :WB], AF.Identity,
                                     bias=half[:], scale=inv[:])
            # i16 clips on DVE (2x rate), f32 affines, store on the Sync
            # HWDGE queue so stores overlap the tail of the s1 loads on Q0
            nc.vector.tensor_scalar(ia[:], ia[:], 8, -7, OP.min, OP.max)
            nc.vector.tensor_scalar(ib[:], ib[:], 8, -7, OP.min, OP.max)
            orow = op_.tile([128, R], f32, tag="orow")
            nc.vector.tensor_scalar(orow[:, 0:WA], ia[:], step[:], hstep[:],
                                    OP.mult, OP.subtract)
            nc.vector.tensor_scalar(orow[:, WA:R], ib[:], step[:], hstep[:],
                                    OP.mult, OP.subtract)
            nc.sync.dma_start(
                y[s * C + b * 128:s * C + (b + 1) * 128, :], orow[:])

        # ---- input side sample 0 ----
        for b in range(NB):
            sq_a(0, b)
            sq_b(0, b)
        sample_scalars(0)
        # ---- phase B s0 (priority-ahead of s1 squares) ----
        phase_b_row(0, 0)
        phase_b_row(0, 1)
        phase_b_row(0, 2)
        # tot(1) PSUM slot reserved mid-rotation so the s1 scalar chain is
        # not gated behind the whole s0 drain pipeline
        tot1 = pp.tile([128, WA], f32, tag="pchunk")
        phase_b_row(0, 3)
        # ---- s1 squares fill ACT/DVE gaps during s0 phase B ----
        for b in range(NB):
            sq_a(1, b)
            sq_b(1, b)
        sample_scalars(1, tot_tile=tot1)
        for b in range(NB):
            phase_b_row(1, b)
    nc.compile()
    return nc


def _get_program():
    if "nc" not in _CACHE:
        _CACHE["nc"] = _build_program()
    return _CACHE["nc"]


def kernel(x: np.ndarray, aux_matrix: np.ndarray) -> np.ndarray:
    from concourse.bass_utils import run_bass_kernel_spmd

    x = np.ascontiguousarray(x, dtype=np.float32)
    aux = np.ascontiguousarray(aux_matrix, dtype=np.float32)
    w128 = np.zeros((128, 128), dtype=np.float32)
    w128[:64, :64] = aux
    w128[64:, 64:] = aux

    nc = _get_program()
    in_maps = [
        {"x": x[c * S:(c + 1) * S].reshape(S * C, R), "w": w128}
        for c in range(NCORES)
    ]
    res = run_bass_kernel_spmd(nc, in_maps, list(range(NCORES)))
    out = np.empty((B, C, HH, WW), dtype=np.float32)
    for c in range(NCORES):
        out[c * S:(c + 1) * S] = res.results[c]["y"].reshape(S, C, HH, WW)
    return out


# revision 34
# speedup vs baseline: 1.1247x; 1.1247x over previous
"""Trainium2 Bass kernel for HalfHadamardTrustQuantizer.

Computation (forward value of the reference, which collapses to xq):
  x_had = blockwise-64 Hadamard rotation of channels:  (B,C,H,W), C=512 = 8 groups of 64
  std   = sqrt(mean(x_had^2)) per sample  (== RMS of x by orthogonality)
  scale = OPT*std + 1e-8 ; step = 2*scale/15
  xq    = round(clip(x_had,-scale,scale)/step + 0.5)*step - step/2

Sharding: data-parallel over batch; 2 samples per core on 8 cores.

Pipeline (per core, 2 samples of (512, 3136) fp32; ~78us vs 113us
baseline; wire-limited: 25.7MB on one DMA queue at ~420-430 GB/s):
  ALL x/y DMA on the single Pool SWDGE queue, strict FIFO: 16 casting
  half-row loads (f32 -> f32r, sample 0 first, split at col 2048 to
  match PSUM drain tiles) followed by 8 full-row casting stores
  (bf16 -> f32).  One stream sustains ~420-430 GB/s; stores begin
  right as loads end; wt loads on the idle Sync HWDGE queue + a DVE
  convert.  A dummy [128,1] Sqrt is the first ACT op so the act-table
  pass pins the sqrt_and_others table (square+identity+sqrt) - no
  1.3us table reloads mid-kernel.
  Input side: ACT squares the 2048-halves, DVE squares the 1088-halves
  (accum_out partial columns).  Scalars: DVE reduce + PE ones-matmul
  broadcast + ACT sqrt + DVE ops; sample 1's tot PSUM slot is reserved
  mid-rotation so its scalar chain is not gated by the full s0 drain
  pipeline.
  Phase B per row: PE f32r matmuls into 2 PSUM tiles (2048/1088); ACT
  drains the 2048 tile (Identity, scale=1/step, bias=0.5 -> i16 RNE),
  the 1088 tile drains on DVE (s0 rows) / ACT (s1 rows); DVE affine
  i16 -> bf16 (2x rate), then ONE full-row bf16 clip on DVE against
  +-scale AP bounds (clip(i,-7,8)*step-step/2 == clip(i*step-step/2,
  -scale,scale), so the int clip becomes a value clip after the
  affine); full-row casting store bf16 -> f32 on Q0.  Pool runs no
  elementwise work - only DMA issues - keeping the drain->store chain
  on ACT+DVE.  Sample 1's squares are emitted after sample 0's phase B
  so the scheduler fills engine gaps with them instead of blocking the
  chain.  bf16 output costs ~0.1% extra rounding (budget 2e-2).
"""

import numpy as np
from contextlib import ExitStack

B, C, HH, WW = 16, 512, 56, 56
R = HH * WW            # 3136 spatial
NCORES = 8
S = B // NCORES        # samples per core
NB = C // 128          # block-rows per sample
N_ELEM = C * R         # per-sample reduction size
OPT = 2.513930578568423
INV_N = np.float64(1.0) / np.float64(N_ELEM)
TWO_15 = np.float32(2.0) / np.float32(15.0)

WA = 2048              # A-half width (cols 0:2048)
WB = R - WA            # B-half width (cols 2048:3136) = 1088

_CACHE = {}


def _build_program():
    import concourse.bacc as bacc
    import concourse.tile as tile
    import concourse.mybir as mybir

    AF = mybir.ActivationFunctionType
    OP = mybir.AluOpType
    f32 = mybir.dt.float32
    f32r = mybir.dt.float32r
    bf16 = mybir.dt.bfloat16
    i16 = mybir.dt.int16

    nc = bacc.Bacc("TRN2", target_bir_lowering=False, debug=False,
                   num_devices=NCORES)
    x = nc.dram_tensor("x", [S * C, R], f32, kind="ExternalInput").ap()
    w = nc.dram_tensor("w", [128, 128], f32, kind="ExternalInput").ap()
    y = nc.dram_tensor("y", [S * C, R], f32, kind="ExternalOutput").ap()

    with tile.TileContext(nc) as tc, ExitStack() as ctx:
        xap = ctx.enter_context(tc.tile_pool(name="xap", bufs=2 * NB))
        xbp = ctx.enter_context(tc.tile_pool(name="xbp", bufs=2 * NB))
        cn = ctx.enter_context(tc.tile_pool(name="cn", bufs=1))
        sq = ctx.enter_context(tc.tile_pool(name="sq", bufs=1))
        ac = ctx.enter_context(tc.tile_pool(name="ac", bufs=2))
        sc = ctx.enter_context(tc.tile_pool(name="sc", bufs=2))
        iap = ctx.enter_context(tc.tile_pool(name="iap", bufs=3))
        ibp = ctx.enter_context(tc.tile_pool(name="ibp", bufs=3))
        op_ = ctx.enter_context(tc.tile_pool(name="op", bufs=3))
        pp = ctx.enter_context(tc.tile_pool(name="pp", bufs=2, space="PSUM"))

        # constants first (instant readiness for the dummy sqrt)
        ones = cn.tile([128, 128], f32, tag="ones")
        nc.gpsimd.memset(ones[:], 1.0)
        half = cn.tile([128, 1], f32, tag="half")
        nc.gpsimd.memset(half[:], 0.5)
        # dummy sqrt pins the sqrt_and_others act table (square+identity+sqrt)
        dum = cn.tile([128, 1], f32, tag="dum")
        nc.scalar.activation(dum[:], half[:], AF.Sqrt)

        # ---- wt on the idle Sync HWDGE queue; DVE converts to f32r ----
        wt_f = cn.tile([128, 128], f32, tag="wf")
        nc.sync.dma_start(wt_f[:], w[:])
        wt = cn.tile([128, 128], f32r, tag="w")
        nc.vector.tensor_scalar_mul(wt[:], wt_f[:], 1.0)
        # ---- Q0 SWDGE FIFO: 16 casting half-loads first, then stores ----

        xas = {}
        xbs = {}
        for s in range(S):
            for b in range(NB):
                r0 = s * C + b * 128
                xa = xap.tile([128, WA], f32r, tag="xa")
                nc.gpsimd.dma_start(xa[:], x[r0:r0 + 128, 0:WA])
                xas[(s, b)] = xa
                xb = xbp.tile([128, WB], f32r, tag="xb")
                nc.gpsimd.dma_start(xb[:], x[r0:r0 + 128, WA:R])
                xbs[(s, b)] = xb

        sqa = sq.tile([128, WA], f32, tag="sqa")   # ACT square scratch
        sqb = sq.tile([128, WB], f32, tag="sqb")   # DVE square scratch

        parts = {}
        for s in range(S):
            parts[s] = ac.tile([128, 2 * NB], f32, tag=f"part{s}",
                               name=f"part{s}")

        scal = {}

        def sq_a(s, b):
            nc.scalar.activation(sqa[:], xas[(s, b)][:].bitcast(f32),
                                 AF.Square,
                                 accum_out=parts[s][:, 2 * b:2 * b + 1])

        def sq_b(s, b):
            src = xbs[(s, b)][:].bitcast(f32)
            nc.vector.scalar_tensor_tensor(sqb[:], src, 1.0, src,
                                           OP.mult, OP.mult,
                                           accum_out=parts[s][:, 2 * b + 1:
                                                              2 * b + 2])

        def sample_scalars(s, tot_tile=None):
            red = sc.tile([128, 1], f32, tag="red")
            nc.vector.reduce_sum(red[:], parts[s][:], axis=mybir.AxisListType.X)
            tot = tot_tile if tot_tile is not None else pp.tile(
                [128, WA], f32, tag="pchunk")
            tot = tot[:, 0:1]
            nc.tensor.matmul(tot[:], ones[:], red[:], start=True, stop=True)
            std = sc.tile([128, 1], f32, tag="std")
            nc.scalar.activation(std[:], tot[:], AF.Sqrt, scale=float(INV_N))
            scale_t = sc.tile([128, 1], f32, tag="scale")
            nc.vector.tensor_scalar(scale_t[:], std[:], float(OPT), 1e-8,
                                    OP.mult, OP.add)
            step = sc.tile([128, 1], f32, tag="step")
            nc.vector.tensor_scalar_mul(step[:], scale_t[:], float(TWO_15))
            inv = sc.tile([128, 1], f32, tag="inv")
            nc.vector.reciprocal(inv[:], step[:])
            hstep = sc.tile([128, 1], f32, tag="hstep")
            nc.vector.tensor_scalar_mul(hstep[:], step[:], 0.5)
            nscale = sc.tile([128, 1], f32, tag="nscale")
            nc.vector.tensor_scalar_mul(nscale[:], scale_t[:], -1.0)
            scal[s] = (inv, step, hstep, scale_t, nscale)

        def phase_b_row(s, b):
            inv, step, hstep, scale_t, nscale = scal[s]
            xa = xas.pop((s, b))
            xb = xbs.pop((s, b))
            pma = pp.tile([128, WA], f32, tag="pchunk")
            for co in range(0, WA, 512):
                nc.tensor.matmul(pma[:, co:co + 512], wt[:],
                                 xa[:, co:co + 512], start=True, stop=True)
            pmb = pp.tile([128, WA], f32, tag="pchunk")
            for co in range(0, WB, 512):
                ch = min(512, WB - co)
                nc.tensor.matmul(pmb[:, co:co + ch], wt[:],
                                 xb[:, co:co + ch], start=True, stop=True)
            ia = iap.tile([128, WA], i16, tag="ia")
            nc.scalar.activation(ia[:], pma[:, :WA], AF.Identity,
                                 bias=half[:], scale=inv[:])
            ib = ibp.tile([128, WB], i16, tag="ib")
            if s == 0:
                nc.vector.tensor_scalar(ib[:], pmb[:, :WB], inv[:], half[:],
                                        OP.mult, OP.add)
            else:
                nc.scalar.activation(ib[:], pmb[:, :WB], AF.Identity,
                                     bias=half[:], scale=inv[:])
            orow = op_.tile([128, R], bf16, tag="orow")
            nc.vector.tensor_scalar(orow[:, 0:WA], ia[:], step[:], hstep[:],
                                    OP.mult, OP.subtract)
            nc.vector.tensor_scalar(orow[:, WA:R], ib[:], step[:], hstep[:],
                                    OP.mult, OP.subtract)
            nc.vector.tensor_scalar(orow[:], orow[:], scale_t[:], nscale[:],
                                    OP.min, OP.max)
            nc.gpsimd.dma_start(
                y[s * C + b * 128:s * C + (b + 1) * 128, :], orow[:])

        # ---- input side sample 0 ----
        for b in range(NB):
            sq_a(0, b)
            sq_b(0, b)
        sample_scalars(0)
        # ---- phase B s0 (priority-ahead of s1 squares) ----
        phase_b_row(0, 0)
        phase_b_row(0, 1)
        phase_b_row(0, 2)
        # tot(1) PSUM slot reserved mid-rotation so the s1 scalar chain is
        # not gated behind the whole s0 drain pipeline
        tot1 = pp.tile([128, WA], f32, tag="pchunk")
        phase_b_row(0, 3)
        # ---- s1 squares fill ACT/DVE gaps during s0 phase B ----
        for b in range(NB):
            sq_a(1, b)
            sq_b(1, b)
        sample_scalars(1, tot_tile=tot1)
        for b in range(NB):
            phase_b_row(1, b)
    nc.compile()
    return nc


def _get_program():
    if "nc" not in _CACHE:
        _CACHE["nc"] = _build_program()
    return _CACHE["nc"]


def kernel(x: np.ndarray, aux_matrix: np.ndarray) -> np.ndarray:
    from concourse.bass_utils import run_bass_kernel_spmd

    x = np.ascontiguousarray(x, dtype=np.float32)
    aux = np.ascontiguousarray(aux_matrix, dtype=np.float32)
    w128 = np.zeros((128, 128), dtype=np.float32)
    w128[:64, :64] = aux
    w128[64:, 64:] = aux

    nc = _get_program()
    in_maps = [
        {"x": x[c * S:(c + 1) * S].reshape(S * C, R), "w": w128}
        for c in range(NCORES)
    ]
    res = run_bass_kernel_spmd(nc, in_maps, list(range(NCORES)))
    out = np.empty((B, C, HH, WW), dtype=np.float32)
    for c in range(NCORES):
        out[c * S:(c + 1) * S] = res.results[c]["y"].reshape(S, C, HH, WW)
    return out
